# revision 18
# baseline (speedup 1.0000x reference)
"""Trainium2 Bass kernel for nn_AlignModel.

Computes out[b, j, i] = sigmoid(simp[b,j]·w_s + orig[b,i]·w_o + bias) where
orig/simp are the two halves of prop_state[b] ([B, 2S, D] -> [B,S,D] each),
w_o = W[0,:D], w_s = W[0,D:].

Sharding: data-parallel over batch B=8 across the 8 NeuronCores.  Host-side
staging per core (layout only -- all compute is on device):
  xot  [512, 2048] f16  = orig(b).T          (d-major, so PE can contract d)
  xs   [2048, 512] f16  = simp(b)
  wrep [128, 4, 128] f16: wrep[k,e,m] = w_o[e*128+k]  (stationary replicated
        along the PE output dim -> matmul broadcasts s_o to all partitions)
  wsbc [128, 512]  f16  = w_s replicated across partitions
  out  [2048, 2048] f16, host-upcast to f32.

The 2e-2 rel-err gate admits fp16 end to end (~1.5e-3 measured).
Per-core HBM traffic: 4.45 MiB in + 8.39 MiB out.

Engine schedule (evolved over ~8 profiled iterations):
  - PE: psum_so[p,i] = b + sum_d w_o[d]*orig[i,d] via 4 bias seeds + 16
    K=128/N=512 fp16 matmuls (w_rep stationary): the matmul does the
    reduction AND the 128-row broadcast, so s_o never materializes.
  - Row-blocks 0..9 ("ScalarE path"): sigmoid ACTIVATE [128,2048]
    PSUM->SBUF f16, bias port adds s_s[t*128+p]; perfectly dense 2us
    cadence.  Simp rows for these load partition-inner (bias columns
    drop straight out of the batched DVE mul+reduce).
  - Row-blocks 10..15 ("DVE path"): sigmoid(s) = 1/(1 + e^-(s_o+b)*e^-s_s).
    Their simp rows load partition-OUTER (4KB descriptor lines, first in
    the queue) -> DVE dots -> tiny scatter to a natural-order row ->
    ScalarE exps it (in its idle window) to es_row f16; eo_row comes from
    one exp over PSUM row 0.  PE refills one [128,2048] PSUM tile with
    1 + E_s (x) E_o per block (grouped stationaries); DVE runs one
    reciprocal_approx_fast per block; SWDGE (gpsimd queue) stores the f32
    result with a cast to f16 in the DMA, so DVE never touches it again.
  - ACT table preloaded by a dep-free dummy sigmoid at t~0.
"""

import numpy as np

import concourse.mybir as mybir
from concourse import bacc, bass_utils
from concourse.tile import TileContext

P = 128          # partitions
D = 512          # feature dim
S = 2048         # sents
NT = S // P      # 16 row-tiles
NE = D // P      # 4 contraction chunks
NOFF = 6         # row-tiles 10..15 take the DVE/recip path
NSCA = NT - NOFF # row-tiles 0..9 take the ScalarE/sigmoid path
SCH = 4          # simp tiles per column-layout load group
OGROUPS = [1, 1, 2, 4, 1, 1]         # sync-store groups over tiles 0..9
NCORES = 8
F32 = mybir.dt.float32
F16 = mybir.dt.float16


def _kernel_body(tc, out, xot, xs, wrep, wsbc, bvec):
    nc = tc.nc
    # simp rows for the ScalarE path, partition-inner: j = n*P + p
    xs_re = xs.rearrange("(n p) d -> p n d", p=P)
    # simp rows for the DVE path, partition-outer: j = NSCA*P + p*NOFF + c
    xso_re = xs[NSCA * P:S, :].rearrange("(p c) d -> p c d", c=NOFF)

    with (
        tc.tile_pool(name="consts", bufs=1) as cpool,
        tc.tile_pool(name="xin", bufs=1) as xpool,
        tc.tile_pool(name="scratch", bufs=3) as spool,
        tc.tile_pool(name="recb", bufs=2) as rpool,
        tc.tile_pool(name="outbuf", bufs=1) as opool,
        tc.tile_pool(name="psum", bufs=1, space="PSUM") as ppool,
    ):
        # preload the sigmoid ACT table while DMAs run: dummy activation
        # whose only deps are two DVE memsets, so it issues almost at t=0.
        dummy = cpool.tile([1, 1], F32, tag="dummy")
        dummy_b = cpool.tile([1, 1], F32, tag="dummyb")
        nc.vector.memset(dummy, 0.0)
        nc.vector.memset(dummy_b, 0.0)
        nc.scalar.activation(dummy, dummy,
                             mybir.ActivationFunctionType.Sigmoid,
                             bias=dummy_b[:, 0:1])

        # tiny loads on the scalar HWDGE queue (empty early)
        wrep_sb = cpool.tile([P, NE, P], F16, tag="wrep")
        wsbc_sb = cpool.tile([P, D], F16, tag="wsbc")
        b_sb = cpool.tile([1, 1], F32, tag="bsb")
        nc.scalar.dma_start(out=wrep_sb, in_=wrep)
        nc.scalar.dma_start(out=wsbc_sb, in_=wsbc)
        nc.scalar.dma_start(out=b_sb, in_=bvec)

        ones_row = cpool.tile([1, P], F16, tag="ones")
        nc.vector.memset(ones_row, 1.0)
        b_row = cpool.tile([1, 512], F16, tag="brow")
        nc.vector.memset(b_row, 0.0)
        nc.vector.tensor_scalar_add(b_row, b_row, b_sb)
        ones2k = cpool.tile([1, S], F16, tag="ones2k")
        nc.gpsimd.memset(ones2k, 1.0)

        # --- input stream (sync queue, FIFO):
        #   xs_off (DVE-path rows, fast 4KB lines, needed first by DVE)
        #   -> xs g0 -> xot e0..3 -> xs g1 -> xs g2(half) ---
        xs_off = xpool.tile([P, NOFF, D], F16, tag="xsoff")
        nc.sync.dma_start(out=xs_off, in_=xso_re)
        xs_all = xpool.tile([P, NSCA, D], F16, tag="xs")
        nc.sync.dma_start(out=xs_all[:, 0:SCH, :], in_=xs_re[:, 0:SCH, :])
        xot_t = []
        for e in range(NE):
            xt = xpool.tile([P, S], F16, tag=f"xot{e}", name=f"xot{e}")
            nc.sync.dma_start(out=xt, in_=xot[e * P:(e + 1) * P, :])
            xot_t.append(xt)
        nc.sync.dma_start(out=xs_all[:, SCH:2 * SCH, :],
                          in_=xs_re[:, SCH:2 * SCH, :])
        nc.sync.dma_start(out=xs_all[:, 2 * SCH:NSCA, :],
                          in_=xs_re[:, 2 * SCH:NSCA, :])

        s_sb_mat = cpool.tile([P, NSCA], F32, tag="ssmat")  # s_s, col t
        s4_mat = cpool.tile([P, NOFF], F32, tag="s4mat")    # DVE-path s_s
        es_rowf = cpool.tile([1, NOFF * P], F32, tag="esrowf")
        es_row = cpool.tile([1, NOFF * P], F16, tag="esrow")
        eo_row = cpool.tile([1, S], F16, tag="eorow")       # e^-(s_o+b)
        sob_psum = ppool.tile([P, S], F32, tag="sob")       # b + s_o
        pt_psum = ppool.tile([P, S], F32, tag="pt")         # 1 + Es*Eo

        # --- PE: b seed, then accumulate w_o-weighted transposed orig ---
        for j in range(S // 512):
            nc.tensor.matmul(sob_psum[:, j * 512:(j + 1) * 512], ones_row,
                             b_row, start=True, stop=False)
        for e in range(NE):
            for j in range(S // 512):
                nc.tensor.matmul(sob_psum[:, j * 512:(j + 1) * 512],
                                 wrep_sb[:, e, :],
                                 xot_t[e][:, j * 512:(j + 1) * 512],
                                 start=False, stop=(e == NE - 1))

        # --- DVE: dots for the DVE-path rows first (gates the recip
        # pipeline), then ScalarE-path groups in consumption order ---
        prodo = spool.tile([P, NOFF, D], F16, tag="prodo")
        for c in range(NOFF):
            nc.vector.tensor_mul(out=prodo[:, c, :], in0=xs_off[:, c, :],
                                 in1=wsbc_sb)
        nc.vector.tensor_reduce(
            s4_mat, prodo, axis=mybir.AxisListType.X, op=mybir.AluOpType.add)
        # scatter to natural-order row: es_rowf[0, p*NOFF+c] = s_s[j0+...]
        nc.scalar.dma_start(
            out=es_rowf.rearrange("o (p c) -> o p c", c=NOFF),
            in_=s4_mat)

        def simp_dots(g, lo, hi):
            prod = spool.tile([P, hi - lo, D], F16, tag="prod",
                              name=f"ps{g}")
            for blk in range(hi - lo):
                nc.vector.tensor_mul(out=prod[:, blk, :],
                                     in0=xs_all[:, lo + blk, :],
                                     in1=wsbc_sb)
            nc.vector.tensor_reduce(
                s_sb_mat[:, lo:hi], prod,
                axis=mybir.AxisListType.X, op=mybir.AluOpType.add)

        simp_dots(0, 0, SCH)
        simp_dots(1, SCH, 2 * SCH)
        simp_dots(2, 2 * SCH, NSCA)

        # --- ScalarE: es exp (idle window), eo exp, then the sigmoids ---
        nc.scalar.activation(es_row, es_rowf,
                             mybir.ActivationFunctionType.Exp, scale=-1.0)
        nc.scalar.activation(eo_row, sob_psum[0:1, :],
                             mybir.ActivationFunctionType.Exp, scale=-1.0)

        group_of_tile = []
        for gi, gsz in enumerate(OGROUPS):
            group_of_tile += [gi] * gsz
        group_start = np.cumsum([0] + OGROUPS).tolist()

        out_all = opool.tile([P, NSCA, S], F16, tag="oall")
        for t in range(NSCA):
            nc.scalar.activation(
                out_all[:, t, :], sob_psum,
                mybir.ActivationFunctionType.Sigmoid,
                bias=s_sb_mat[:, t:t + 1], scale=1.0)
            gi = group_of_tile[t]
            if t == group_start[gi] + OGROUPS[gi] - 1:
                t0_g = group_start[gi]
                gsz = OGROUPS[gi]
                r0 = t0_g * P
                if gsz == 1:
                    nc.sync.dma_start(out=out[r0:r0 + P, :],
                                      in_=out_all[:, t0_g, :])
                else:
                    dst = out[r0:r0 + gsz * P, :].rearrange(
                        "(q p) i -> p q i", p=P)
                    nc.sync.dma_start(out=dst,
                                      in_=out_all[:, t0_g:t0_g + gsz, :])

        # --- DVE path for tiles 10..15: PE fills 1 + Es (x) Eo (grouped
        # stationaries: 4 seeds then 4 accums), DVE reciprocal, SWDGE
        # stores f32 -> f16 with the cast inside the DMA ---
        for k in range(NOFF):
            t = NSCA + k
            for j in range(S // 512):
                nc.tensor.matmul(pt_psum[:, j * 512:(j + 1) * 512],
                                 ones_row, ones2k[:, j * 512:(j + 1) * 512],
                                 start=True, stop=False)
            for j in range(S // 512):
                nc.tensor.matmul(pt_psum[:, j * 512:(j + 1) * 512],
                                 es_row[:, k * P:(k + 1) * P],
                                 eo_row[:, j * 512:(j + 1) * 512],
                                 start=False, stop=True)
            srecip = rpool.tile([P, S], F32, tag="srecip", name=f"sr{k}")
            nc.vector.reciprocal_approx_fast(out=srecip, in_=pt_psum)
            nc.gpsimd.dma_start(out=out[t * P:(t + 1) * P, :], in_=srecip)


def build_program():
    nc = bacc.Bacc(
        "TRN2",
        debug=False,
        target_bir_lowering=False,
        num_devices=NCORES,
    )
    xot = nc.dram_tensor("xot", [D, S], F16, kind="ExternalInput").ap()
    xs = nc.dram_tensor("xs", [S, D], F16, kind="ExternalInput").ap()
    wrep = nc.dram_tensor("wrep", [P, NE, P], F16, kind="ExternalInput").ap()
    wsbc = nc.dram_tensor("wsbc", [P, D], F16, kind="ExternalInput").ap()
    bvec = nc.dram_tensor("bvec", [1, 1], F32, kind="ExternalInput").ap()
    out = nc.dram_tensor("out", [S, S], F16, kind="ExternalOutput").ap()
    with TileContext(nc) as tc:
        _kernel_body(tc, out, xot, xs, wrep, wsbc, bvec)
    nc.compile()
    return nc


_PROGRAM = None


def _get_program():
    global _PROGRAM
    if _PROGRAM is None:
        _PROGRAM = build_program()
    return _PROGRAM


def make_in_maps(prop_state, W, b):
    prop = np.asarray(prop_state, dtype=np.float32).astype(np.float16)
    w = np.asarray(W, dtype=np.float32).reshape(2 * D).astype(np.float16)
    w_o, w_s = w[:D], w[D:]
    # wrep[k, e, m] = w_o[e*128 + k], replicated along m (PE output dim)
    wrep = np.ascontiguousarray(
        np.broadcast_to(w_o.reshape(NE, P).T[:, :, None], (P, NE, P)))
    wsbc = np.ascontiguousarray(np.broadcast_to(w_s[None, :], (P, D)))
    bv = np.ascontiguousarray(np.asarray(b, dtype=np.float32).reshape(1, 1))
    maps = []
    for i in range(NCORES):
        xot = np.ascontiguousarray(prop[i, :S].T)         # [512, 2048]
        xs = np.ascontiguousarray(prop[i, S:])            # [2048, 512]
        maps.append({"xot": xot, "xs": xs, "wrep": wrep,
                     "wsbc": wsbc, "bvec": bv})
    return maps


def kernel(A, prop_state, W, b, _trace=False):
    nc = _get_program()
    in_maps = make_in_maps(prop_state, W, b)
    res = bass_utils.run_bass_kernel_spmd(
        nc, in_maps, core_ids=list(range(NCORES)), trace=_trace)
    out = np.stack([res.results[i]["out"] for i in range(NCORES)], axis=0)
    if _trace:
        kernel.last_results = res
    return out.astype(np.float32)


# revision 22
# speedup vs baseline: 1.0147x; 1.0147x over previous
"""Trainium2 Bass kernel for nn_AlignModel.

Computes out[b, j, i] = sigmoid(simp[b,j]·w_s + orig[b,i]·w_o + bias) where
orig/simp are the two halves of prop_state[b] ([B, 2S, D] -> [B,S,D] each),
w_o = W[0,:D], w_s = W[0,D:].

Sharding: data-parallel over batch B=8 across the 8 NeuronCores.  Host-side
staging per core (layout only -- all compute is on device):
  xot  [512, 2048] f16  = orig(b).T          (d-major, so PE can contract d)
  xs   [2048, 512] f16  = simp(b)
  wrep [128, 4, 128] f16: wrep[k,e,m] = w_o[e*128+k]  (stationary replicated
        along the PE output dim -> matmul broadcasts s_o to all partitions)
  wsbc [128, 512]  f16  = w_s replicated across partitions
  out  [2048, 2048] f16, host-upcast to f32.

The 2e-2 rel-err gate admits fp16 end to end (~1.5e-3 measured).
Per-core HBM traffic: 4.45 MiB in + 8.39 MiB out.

Engine schedule (evolved over ~8 profiled iterations):
  - PE: psum_so[p,i] = b + sum_d w_o[d]*orig[i,d] via 4 bias seeds + 16
    K=128/N=512 fp16 matmuls (w_rep stationary): the matmul does the
    reduction AND the 128-row broadcast, so s_o never materializes.
  - Row-blocks 0..9 ("ScalarE path"): sigmoid ACTIVATE [128,2048]
    PSUM->SBUF f16, bias port adds s_s[t*128+p]; perfectly dense 2us
    cadence.  Simp rows for these load partition-inner (bias columns
    drop straight out of the batched DVE mul+reduce).
  - Row-blocks 10..15 ("DVE path"): sigmoid(s) = 1/(1 + e^-(s_o+b)*e^-s_s).
    Their simp rows load partition-OUTER (4KB descriptor lines, first in
    the queue) -> DVE dots -> tiny scatter to a natural-order row ->
    ScalarE exps it (in its idle window) to es_row f16; eo_row comes from
    one exp over PSUM row 0.  PE refills one [128,2048] PSUM tile with
    1 + E_s (x) E_o per block (grouped stationaries); DVE runs one
    reciprocal_approx_fast per block; SWDGE (gpsimd queue) stores the f32
    result with a cast to f16 in the DMA, so DVE never touches it again.
  - ACT table preloaded by a dep-free dummy sigmoid at t~0.
"""

import numpy as np

import concourse.mybir as mybir
from concourse import bacc, bass_utils
from concourse.tile import TileContext

P = 128          # partitions
D = 512          # feature dim
S = 2048         # sents
NT = S // P      # 16 row-tiles
NE = D // P      # 4 contraction chunks
NOFF = 6         # row-tiles 10..15 take the DVE/recip path
NSCA = NT - NOFF # row-tiles 0..9 take the ScalarE/sigmoid path
SCH = 4          # simp tiles per column-layout load group
OGROUPS = [1, 1, 2, 4, 1, 1]         # sync-store groups over tiles 0..9
NCORES = 8
F32 = mybir.dt.float32
F16 = mybir.dt.float16


def _kernel_body(tc, out, xot, xs, wrep, wsbc, bvec):
    nc = tc.nc
    # simp rows for the ScalarE path, partition-inner: j = n*P + p
    xs_re = xs.rearrange("(n p) d -> p n d", p=P)
    # simp rows for the DVE path, partition-outer: j = NSCA*P + p*NOFF + c
    xso_re = xs[NSCA * P:S, :].rearrange("(p c) d -> p c d", c=NOFF)

    with (
        tc.tile_pool(name="consts", bufs=1) as cpool,
        tc.tile_pool(name="xin", bufs=1) as xpool,
        tc.tile_pool(name="scratch", bufs=3) as spool,
        tc.tile_pool(name="recb", bufs=2) as rpool,
        tc.tile_pool(name="outbuf", bufs=1) as opool,
        tc.tile_pool(name="psum", bufs=1, space="PSUM") as ppool,
    ):
        # preload the sigmoid ACT table while DMAs run: dummy activation
        # whose only deps are two DVE memsets, so it issues almost at t=0.
        dummy = cpool.tile([1, 1], F32, tag="dummy")
        dummy_b = cpool.tile([1, 1], F32, tag="dummyb")
        nc.vector.memset(dummy, 0.0)
        nc.vector.memset(dummy_b, 0.0)
        nc.scalar.activation(dummy, dummy,
                             mybir.ActivationFunctionType.Sigmoid,
                             bias=dummy_b[:, 0:1])

        # tiny loads on the scalar HWDGE queue (empty early)
        wrep_sb = cpool.tile([P, NE, P], F16, tag="wrep")
        wsbc_sb = cpool.tile([P, D], F16, tag="wsbc")
        b_sb = cpool.tile([1, 1], F32, tag="bsb")
        nc.scalar.dma_start(out=wrep_sb, in_=wrep)
        nc.scalar.dma_start(out=wsbc_sb, in_=wsbc)
        nc.scalar.dma_start(out=b_sb, in_=bvec)

        ones_row = cpool.tile([1, P], F16, tag="ones")
        nc.vector.memset(ones_row, 1.0)
        b_row = cpool.tile([1, 512], F16, tag="brow")
        nc.vector.memset(b_row, 0.0)
        nc.vector.tensor_scalar_add(b_row, b_row, b_sb)
        # [es; 1] and [eo; 1] operand pairs on adjacent partitions: one K=2
        # matmul then computes 1 + Es (x) Eo straight into PSUM.  Row 1 is
        # memset to 1.0 up front; ScalarE's exps overwrite row 0 later.
        es_ones = cpool.tile([2, NOFF * P], F16, tag="esones")
        eo_ones = cpool.tile([2, S], F16, tag="eoones")
        nc.gpsimd.memset(es_ones, 1.0)
        nc.gpsimd.memset(eo_ones, 1.0)

        # --- input stream (sync queue, FIFO):
        #   xs_off (DVE-path rows, fast 4KB lines, needed first by DVE)
        #   -> xs g0 -> xot e0..3 -> xs g1 -> xs g2(half) ---
        xs_off = xpool.tile([P, NOFF, D], F16, tag="xsoff")
        nc.sync.dma_start(out=xs_off, in_=xso_re)
        xs_all = xpool.tile([P, NSCA, D], F16, tag="xs")
        nc.sync.dma_start(out=xs_all[:, 0:SCH, :], in_=xs_re[:, 0:SCH, :])
        xot_t = []
        for e in range(NE):
            xt = xpool.tile([P, S], F16, tag=f"xot{e}", name=f"xot{e}")
            nc.sync.dma_start(out=xt, in_=xot[e * P:(e + 1) * P, :])
            xot_t.append(xt)
        nc.sync.dma_start(out=xs_all[:, SCH:2 * SCH, :],
                          in_=xs_re[:, SCH:2 * SCH, :])
        nc.sync.dma_start(out=xs_all[:, 2 * SCH:NSCA, :],
                          in_=xs_re[:, 2 * SCH:NSCA, :])

        s_sb_mat = cpool.tile([P, NSCA], F32, tag="ssmat")  # s_s, col t
        s4_mat = cpool.tile([P, NOFF], F32, tag="s4mat")    # DVE-path s_s
        es_rowf = cpool.tile([1, NOFF * P], F32, tag="esrowf")
        sob_psum = ppool.tile([P, S], F32, tag="sob")       # b + s_o
        # two half-width tiles ping-pong so PE fill overlaps DVE reciprocal
        pt_a = ppool.tile([P, S // 2], F32, tag="pta")
        pt_b = ppool.tile([P, S // 2], F32, tag="ptb")

        # --- PE: b seed, then accumulate w_o-weighted transposed orig ---
        for j in range(S // 512):
            nc.tensor.matmul(sob_psum[:, j * 512:(j + 1) * 512], ones_row,
                             b_row, start=True, stop=False)
        for e in range(NE):
            for j in range(S // 512):
                nc.tensor.matmul(sob_psum[:, j * 512:(j + 1) * 512],
                                 wrep_sb[:, e, :],
                                 xot_t[e][:, j * 512:(j + 1) * 512],
                                 start=False, stop=(e == NE - 1))

        # --- DVE: dots for the DVE-path rows first (gates the recip
        # pipeline), then ScalarE-path groups in consumption order ---
        prodo = spool.tile([P, NOFF, D], F16, tag="prodo")
        for c in range(NOFF):
            nc.vector.tensor_mul(out=prodo[:, c, :], in0=xs_off[:, c, :],
                                 in1=wsbc_sb)
        nc.vector.tensor_reduce(
            s4_mat, prodo, axis=mybir.AxisListType.X, op=mybir.AluOpType.add)
        # scatter to natural-order row: es_rowf[0, p*NOFF+c] = s_s[j0+...].
        # Issued on the sync queue: an ACT-queue issue would head-block the
        # whole ScalarE sigmoid stream behind this DVE reduce.
        nc.sync.dma_start(
            out=es_rowf.rearrange("o (p c) -> o p c", c=NOFF),
            in_=s4_mat)

        def simp_dots(g, lo, hi):
            prod = spool.tile([P, hi - lo, D], F16, tag="prod",
                              name=f"ps{g}")
            for blk in range(hi - lo):
                nc.vector.tensor_mul(out=prod[:, blk, :],
                                     in0=xs_all[:, lo + blk, :],
                                     in1=wsbc_sb)
            nc.vector.tensor_reduce(
                s_sb_mat[:, lo:hi], prod,
                axis=mybir.AxisListType.X, op=mybir.AluOpType.add)

        simp_dots(0, 0, SCH)
        simp_dots(1, SCH, 2 * SCH)
        simp_dots(2, 2 * SCH, NSCA)

        # --- ScalarE: es exp first (its scatter lands before PSUM is done),
        # sigmoids 0-1, eo exp, then the remaining sigmoids ---
        nc.scalar.activation(es_ones[0:1, :], es_rowf,
                             mybir.ActivationFunctionType.Exp, scale=-1.0)

        group_of_tile = []
        for gi, gsz in enumerate(OGROUPS):
            group_of_tile += [gi] * gsz
        group_start = np.cumsum([0] + OGROUPS).tolist()

        out_all = opool.tile([P, NSCA, S], F16, tag="oall")

        def sigmoid_tile(t):
            nc.scalar.activation(
                out_all[:, t, :], sob_psum,
                mybir.ActivationFunctionType.Sigmoid,
                bias=s_sb_mat[:, t:t + 1], scale=1.0)
            gi = group_of_tile[t]
            if t == group_start[gi] + OGROUPS[gi] - 1:
                t0_g = group_start[gi]
                gsz = OGROUPS[gi]
                r0 = t0_g * P
                if gsz == 1:
                    nc.sync.dma_start(out=out[r0:r0 + P, :],
                                      in_=out_all[:, t0_g, :])
                else:
                    dst = out[r0:r0 + gsz * P, :].rearrange(
                        "(q p) i -> p q i", p=P)
                    nc.sync.dma_start(out=dst,
                                      in_=out_all[:, t0_g:t0_g + gsz, :])

        sigmoid_tile(0)
        sigmoid_tile(1)
        nc.scalar.activation(eo_ones[0:1, :], sob_psum[0:1, :],
                             mybir.ActivationFunctionType.Exp, scale=-1.0)
        for t in range(2, NSCA):
            sigmoid_tile(t)

        # --- DVE path for tiles 10..15: one K=2 matmul per 512-col bank
        # computes 1 + Es (x) Eo into half-width PSUM tiles (ping-pong so
        # PE fill overlaps DVE reciprocal); SWDGE stores f32 -> f16 with
        # the cast inside the DMA ---
        H = S // 2
        for k in range(NOFF):
            t = NSCA + k
            srecip = rpool.tile([P, S], F32, tag="srecip", name=f"sr{k}")
            for h, pt in enumerate((pt_a, pt_b)):
                for j in range(H // 512):
                    c0 = h * H + j * 512
                    nc.tensor.matmul(pt[:, j * 512:(j + 1) * 512],
                                     es_ones[:, k * P:(k + 1) * P],
                                     eo_ones[:, c0:c0 + 512],
                                     start=True, stop=True)
                nc.vector.reciprocal_approx_fast(
                    out=srecip[:, h * H:(h + 1) * H], in_=pt)
            nc.gpsimd.dma_start(out=out[t * P:(t + 1) * P, :], in_=srecip)


def build_program():
    nc = bacc.Bacc(
        "TRN2",
        debug=False,
        target_bir_lowering=False,
        num_devices=NCORES,
    )
    xot = nc.dram_tensor("xot", [D, S], F16, kind="ExternalInput").ap()
    xs = nc.dram_tensor("xs", [S, D], F16, kind="ExternalInput").ap()
    wrep = nc.dram_tensor("wrep", [P, NE, P], F16, kind="ExternalInput").ap()
    wsbc = nc.dram_tensor("wsbc", [P, D], F16, kind="ExternalInput").ap()
    bvec = nc.dram_tensor("bvec", [1, 1], F32, kind="ExternalInput").ap()
    out = nc.dram_tensor("out", [S, S], F16, kind="ExternalOutput").ap()
    with TileContext(nc) as tc:
        _kernel_body(tc, out, xot, xs, wrep, wsbc, bvec)
    nc.compile()
    return nc


_PROGRAM = None


def _get_program():
    global _PROGRAM
    if _PROGRAM is None:
        _PROGRAM = build_program()
    return _PROGRAM


def make_in_maps(prop_state, W, b):
    prop = np.asarray(prop_state, dtype=np.float32).astype(np.float16)
    w = np.asarray(W, dtype=np.float32).reshape(2 * D).astype(np.float16)
    w_o, w_s = w[:D], w[D:]
    # wrep[k, e, m] = w_o[e*128 + k], replicated along m (PE output dim)
    wrep = np.ascontiguousarray(
        np.broadcast_to(w_o.reshape(NE, P).T[:, :, None], (P, NE, P)))
    wsbc = np.ascontiguousarray(np.broadcast_to(w_s[None, :], (P, D)))
    bv = np.ascontiguousarray(np.asarray(b, dtype=np.float32).reshape(1, 1))
    maps = []
    for i in range(NCORES):
        xot = np.ascontiguousarray(prop[i, :S].T)         # [512, 2048]
        xs = np.ascontiguousarray(prop[i, S:])            # [2048, 512]
        maps.append({"xot": xot, "xs": xs, "wrep": wrep,
                     "wsbc": wsbc, "bvec": bv})
    return maps


def kernel(A, prop_state, W, b, _trace=False):
    nc = _get_program()
    in_maps = make_in_maps(prop_state, W, b)
    res = bass_utils.run_bass_kernel_spmd(
        nc, in_maps, core_ids=list(range(NCORES)), trace=_trace)
    out = np.stack([res.results[i]["out"] for i in range(NCORES)], axis=0)
    if _trace:
        kernel.last_results = res
    return out.astype(np.float32)


# revision 29
# speedup vs baseline: 1.1034x; 1.0875x over previous
"""Trainium2 Bass kernel for nn_AlignModel.

Computes out[b, j, i] = sigmoid(simp[b,j]·w_s + orig[b,i]·w_o + bias) where
orig/simp are the two halves of prop_state[b] ([B, 2S, D] -> [B,S,D] each),
w_o = W[0,:D], w_s = W[0,D:].

Sharding: data-parallel over batch B=8 across the 8 NeuronCores.  Host-side
staging per core (layout only -- all compute is on device):
  xot  [512, 2048] f16  = orig(b).T          (d-major, so PE can contract d)
  xs   [2048, 512] f16  = simp(b)
  wrep [128, 4, 128] f16: wrep[k,e,m] = w_o[e*128+k]  (stationary replicated
        along the PE output dim -> matmul broadcasts s_o to all partitions)
  wsbc [128, 512]  f16  = w_s replicated across partitions
  id128 [128, 128] f32  = identity (PE-transpose stationary)
  out  [2048, 2048] f16, host-upcast to f32.

The 2e-2 rel-err gate admits fp16 end to end (~1.5e-3 measured).
Per-core HBM traffic: 4.5 MiB in + 8.39 MiB out.

Engine schedule (evolved over ~10 profiled iterations):
  - PE: psum_so[p,i] = b + sum_d w_o[d]*orig[i,d] via 4 bias seeds + 16
    K=128/N=512 fp16 matmuls (w_rep stationary): the matmul does the
    reduction AND the 128-row broadcast, so s_o never materializes.
  - Row-blocks 0..11 (ScalarE): sigmoid ACTIVATE [128,2048] PSUM->SBUF
    f16, bias port adds s_s[t*128+p]; dense ~2us cadence.  Their simp rows
    load partition-inner so bias columns drop out of the DVE mul+reduce.
  - Row-blocks 12..15 (DVE): sigmoid(s) = 1/(1 + e^-(s_o+b)*e^-s_s).
    Their simp rows load partition-OUTER (4KB lines, first in queue); the
    [128,4] s_s matrix is interleaved with zero columns and PE-transposed
    (identity stationary) so after ScalarE's exp the zeros become the
    "ones" rows: each block's PSUM fill 1 + Es (x) Eo is then a single
    K=2 matmul per bank.  Two half-width PSUM tiles ping-pong so PE fill
    overlaps DVE's reciprocal_approx_fast; DVE casts f32->f16 and the
    stores ride the fast sync HWDGE queue (SWDGE cast-stores measured
    ~5us each and stalled the pipeline; a scatter DMA for es cost ~8us).
  - ACT table preloaded by a dep-free dummy sigmoid at t~0.
"""

import numpy as np

import concourse.mybir as mybir
from concourse import bacc, bass_utils
from concourse.tile import TileContext

P = 128          # partitions
D = 512          # feature dim
S = 2048         # sents
NT = S // P      # 16 row-tiles
NE = D // P      # 4 contraction chunks
NOFF = 4         # row-tiles 12..15 take the DVE/recip path
NSCA = NT - NOFF # row-tiles 0..11 take the ScalarE/sigmoid path
OGROUPS = [1, 1, 2, 4, 4]            # sync-store groups over tiles 0..11
NCORES = 8
F32 = mybir.dt.float32
F16 = mybir.dt.float16


def _kernel_body(tc, out, xot, xs, wrep, wsbc, id128, bvec):
    nc = tc.nc
    # simp rows for the ScalarE path, partition-inner: j = n*P + p
    xs_re = xs.rearrange("(n p) d -> p n d", p=P)
    # simp rows for the DVE path, partition-outer: j = NSCA*P + p*NOFF + c
    xso_re = xs[NSCA * P:S, :].rearrange("(p c) d -> p c d", c=NOFF)

    with (
        tc.tile_pool(name="consts", bufs=1) as cpool,
        tc.tile_pool(name="xin", bufs=1) as xpool,
        tc.tile_pool(name="scratch", bufs=3) as spool,
        tc.tile_pool(name="recb", bufs=3) as rpool,
        tc.tile_pool(name="outbuf", bufs=1) as opool,
        tc.tile_pool(name="psum", bufs=1, space="PSUM") as ppool,
    ):
        # preload the sigmoid ACT table while DMAs run: dummy activation
        # whose only deps are two DVE memsets, so it issues almost at t=0.
        dummy = cpool.tile([1, 1], F32, tag="dummy")
        dummy_b = cpool.tile([1, 1], F32, tag="dummyb")
        nc.vector.memset(dummy, 0.0)
        nc.vector.memset(dummy_b, 0.0)
        nc.scalar.activation(dummy, dummy,
                             mybir.ActivationFunctionType.Sigmoid,
                             bias=dummy_b[:, 0:1])

        # tiny loads on the scalar HWDGE queue (empty early)
        wrep_sb = cpool.tile([P, NE, P], F16, tag="wrep")
        wsbc_sb = cpool.tile([P, D], F16, tag="wsbc")
        id_sb = cpool.tile([P, P], F32, tag="id128")
        b_sb = cpool.tile([1, 1], F32, tag="bsb")
        nc.scalar.dma_start(out=wrep_sb, in_=wrep)
        nc.scalar.dma_start(out=wsbc_sb, in_=wsbc)
        nc.scalar.dma_start(out=id_sb, in_=id128)
        nc.scalar.dma_start(out=b_sb, in_=bvec)

        ones_row = cpool.tile([1, P], F16, tag="ones")
        nc.vector.memset(ones_row, 1.0)
        b_row = cpool.tile([1, 512], F16, tag="brow")
        nc.vector.memset(b_row, 0.0)
        nc.vector.tensor_scalar_add(b_row, b_row, b_sb)
        # eo pair [e^-(s_o+b); 1] on adjacent partitions: one K=2 matmul
        # then computes 1 + Es*Eo straight into PSUM.  Row 1 memset now;
        # ScalarE's exp overwrites row 0 later.
        eo_ones = cpool.tile([2, S], F16, tag="eoones")
        nc.gpsimd.memset(eo_ones, 1.0)

        # --- input stream (sync queue, FIFO): xs_off -> xs[0:2] -> xot
        # e0..3 -> xs[2:4] -> xs[4:8] -> xs[8:12] ---
        xs_off = xpool.tile([P, NOFF, D], F16, tag="xsoff")
        nc.sync.dma_start(out=xs_off, in_=xso_re)
        xs_all = xpool.tile([P, NSCA, D], F16, tag="xs")
        nc.sync.dma_start(out=xs_all[:, 0:2, :], in_=xs_re[:, 0:2, :])
        xot_t = []
        for e in range(NE):
            xt = xpool.tile([P, S], F16, tag=f"xot{e}", name=f"xot{e}")
            nc.sync.dma_start(out=xt, in_=xot[e * P:(e + 1) * P, :])
            xot_t.append(xt)
        for lo, hi in ((2, 4), (4, 8), (8, 12)):
            nc.sync.dma_start(out=xs_all[:, lo:hi, :],
                              in_=xs_re[:, lo:hi, :])

        s_sb_mat = cpool.tile([P, NSCA], F32, tag="ssmat")  # s_s, col t
        s4_mat = cpool.tile([P, NOFF], F32, tag="s4mat")    # DVE-path s_s
        s4i = cpool.tile([P, 2 * NOFF], F32, tag="s4i")     # s_s cols | 0
        nc.vector.memset(s4i, 0.0)
        es2f = cpool.tile([2, NOFF * P], F32, tag="es2f")
        es2 = cpool.tile([2, NOFF * P], F16, tag="es2")     # [Es row; 1s]
        sob_psum = ppool.tile([P, S], F32, tag="sob")       # b + s_o
        # two half-width tiles ping-pong so PE fill overlaps DVE reciprocal
        pt_a = ppool.tile([P, S // 2], F32, tag="pta")
        pt_b = ppool.tile([P, S // 2], F32, tag="ptb")

        # --- PE: b seed, then accumulate w_o-weighted transposed orig ---
        for j in range(S // 512):
            nc.tensor.matmul(sob_psum[:, j * 512:(j + 1) * 512], ones_row,
                             b_row, start=True, stop=False)
        for e in range(NE):
            for j in range(S // 512):
                nc.tensor.matmul(sob_psum[:, j * 512:(j + 1) * 512],
                                 wrep_sb[:, e, :],
                                 xot_t[e][:, j * 512:(j + 1) * 512],
                                 start=False, stop=(e == NE - 1))

        # --- DVE: dots for the DVE-path rows first, interleaved with
        # zeros, then PE-transposed; ScalarE exps the result to f16 ---
        prodo = spool.tile([P, NOFF, D], F16, tag="prodo")
        for c in range(NOFF):
            nc.vector.tensor_mul(out=prodo[:, c, :], in0=xs_off[:, c, :],
                                 in1=wsbc_sb)
        nc.vector.tensor_reduce(
            s4_mat, prodo, axis=mybir.AxisListType.X, op=mybir.AluOpType.add)
        nc.vector.tensor_copy(
            out=s4i.rearrange("p (c two) -> p c two", two=2)[:, :, 0],
            in_=s4_mat)
        # four [128,2] -> [2,128] PE transposes (stationary base must be a
        # multiple of 32, so each pair lands at partitions 0-1 of its own
        # PSUM column range), then one DVE copy out
        for k in range(NOFF):
            nc.tensor.transpose(out=pt_a[0:2, k * P:(k + 1) * P],
                                in_=s4i[:, 2 * k:2 * k + 2],
                                identity=id_sb)
        nc.vector.tensor_copy(out=es2f, in_=pt_a[0:2, 0:NOFF * P])

        def simp_dots(g, lo, hi):
            prod = spool.tile([P, hi - lo, D], F16, tag="prod",
                              name=f"ps{g}")
            for blk in range(hi - lo):
                nc.vector.tensor_mul(out=prod[:, blk, :],
                                     in0=xs_all[:, lo + blk, :],
                                     in1=wsbc_sb)
            nc.vector.tensor_reduce(
                s_sb_mat[:, lo:hi], prod,
                axis=mybir.AxisListType.X, op=mybir.AluOpType.add)

        simp_dots(0, 0, 2)
        simp_dots(1, 2, 4)
        simp_dots(2, 4, 8)
        simp_dots(3, 8, 12)

        group_of_tile = []
        for gi, gsz in enumerate(OGROUPS):
            group_of_tile += [gi] * gsz
        group_start = np.cumsum([0] + OGROUPS).tolist()

        out_all = opool.tile([P, NT, S], F16, tag="oall")

        def sigmoid_tile(t):
            nc.scalar.activation(
                out_all[:, t, :], sob_psum,
                mybir.ActivationFunctionType.Sigmoid,
                bias=s_sb_mat[:, t:t + 1], scale=1.0)
            gi = group_of_tile[t]
            if t == group_start[gi] + OGROUPS[gi] - 1:
                t0_g = group_start[gi]
                gsz = OGROUPS[gi]
                r0 = t0_g * P
                if gsz == 1:
                    nc.sync.dma_start(out=out[r0:r0 + P, :],
                                      in_=out_all[:, t0_g, :])
                else:
                    dst = out[r0:r0 + gsz * P, :].rearrange(
                        "(q p) i -> p q i", p=P)
                    nc.sync.dma_start(out=dst,
                                      in_=out_all[:, t0_g:t0_g + gsz, :])

        # ScalarE: sig0, es exp (input ready by then), sig1, eo exp, rest
        sigmoid_tile(0)
        nc.scalar.activation(es2, es2f,
                             mybir.ActivationFunctionType.Exp, scale=-1.0)
        sigmoid_tile(1)
        nc.scalar.activation(eo_ones[0:1, :], sob_psum[0:1, :],
                             mybir.ActivationFunctionType.Exp, scale=-1.0)
        for t in range(2, NSCA):
            sigmoid_tile(t)

        # --- DVE path, tiles 12..15: K=2 matmul fills 1 + Es (x) Eo into
        # ping-pong half-PSUM tiles; DVE reciprocal + f16 cast; stores on
        # the sync queue like everything else ---
        H = S // 2
        for k in range(NOFF):
            t = NSCA + k
            srecip = rpool.tile([P, S], F32, tag="srecip", name=f"sr{k}")
            for h, pt in enumerate((pt_a, pt_b)):
                for j in range(H // 512):
                    c0 = h * H + j * 512
                    nc.tensor.matmul(pt[:, j * 512:(j + 1) * 512],
                                     es2[:, k * P:(k + 1) * P],
                                     eo_ones[:, c0:c0 + 512],
                                     start=True, stop=True)
                nc.vector.reciprocal_approx_fast(
                    out=srecip[:, h * H:(h + 1) * H], in_=pt)
            nc.vector.tensor_copy(out=out_all[:, t, :], in_=srecip)
            nc.sync.dma_start(out=out[t * P:(t + 1) * P, :],
                              in_=out_all[:, t, :])


def build_program():
    nc = bacc.Bacc(
        "TRN2",
        debug=False,
        target_bir_lowering=False,
        num_devices=NCORES,
    )
    xot = nc.dram_tensor("xot", [D, S], F16, kind="ExternalInput").ap()
    xs = nc.dram_tensor("xs", [S, D], F16, kind="ExternalInput").ap()
    wrep = nc.dram_tensor("wrep", [P, NE, P], F16, kind="ExternalInput").ap()
    wsbc = nc.dram_tensor("wsbc", [P, D], F16, kind="ExternalInput").ap()
    id128 = nc.dram_tensor("id128", [P, P], F32, kind="ExternalInput").ap()
    bvec = nc.dram_tensor("bvec", [1, 1], F32, kind="ExternalInput").ap()
    out = nc.dram_tensor("out", [S, S], F16, kind="ExternalOutput").ap()
    with TileContext(nc) as tc:
        _kernel_body(tc, out, xot, xs, wrep, wsbc, id128, bvec)
    nc.compile()
    return nc


_PROGRAM = None


def _get_program():
    global _PROGRAM
    if _PROGRAM is None:
        _PROGRAM = build_program()
    return _PROGRAM


def make_in_maps(prop_state, W, b):
    prop = np.asarray(prop_state, dtype=np.float32).astype(np.float16)
    w = np.asarray(W, dtype=np.float32).reshape(2 * D).astype(np.float16)
    w_o, w_s = w[:D], w[D:]
    # wrep[k, e, m] = w_o[e*128 + k], replicated along m (PE output dim)
    wrep = np.ascontiguousarray(
        np.broadcast_to(w_o.reshape(NE, P).T[:, :, None], (P, NE, P)))
    wsbc = np.ascontiguousarray(np.broadcast_to(w_s[None, :], (P, D)))
    id128 = np.eye(P, dtype=np.float32)
    bv = np.ascontiguousarray(np.asarray(b, dtype=np.float32).reshape(1, 1))
    maps = []
    for i in range(NCORES):
        xot = np.ascontiguousarray(prop[i, :S].T)         # [512, 2048]
        xs = prop[i, S:].copy()                           # [2048, 512]
        # DVE-path rows: permute so the device's partition-outer load
        # (j = j0 + p*NOFF + c) followed by the PE transpose yields
        # s_s[j0 + c*128 + p] at es2[0, c*128+p]
        j0 = NSCA * P
        xs[j0:] = xs[j0:].reshape(NOFF, P, D).transpose(1, 0, 2).reshape(
            NOFF * P, D)
        maps.append({"xot": xot, "xs": np.ascontiguousarray(xs),
                     "wrep": wrep, "wsbc": wsbc, "id128": id128,
                     "bvec": bv})
    return maps


def kernel(A, prop_state, W, b, _trace=False):
    nc = _get_program()
    in_maps = make_in_maps(prop_state, W, b)
    res = bass_utils.run_bass_kernel_spmd(
        nc, in_maps, core_ids=list(range(NCORES)), trace=_trace)
    out = np.stack([res.results[i]["out"] for i in range(NCORES)], axis=0)
    if _trace:
        kernel.last_results = res
    return out.astype(np.float32)


# revision 32
# speedup vs baseline: 1.1098x; 1.0057x over previous
"""Trainium2 Bass kernel for nn_AlignModel.

Computes out[b, j, i] = sigmoid(simp[b,j]·w_s + orig[b,i]·w_o + bias) where
orig/simp are the two halves of prop_state[b] ([B, 2S, D] -> [B,S,D] each),
w_o = W[0,:D], w_s = W[0,D:].

Sharding: data-parallel over batch B=8 across the 8 NeuronCores.  Host-side
staging per core (layout only -- all compute is on device):
  xot  [512, 2048] f16  = orig(b).T          (d-major, so PE can contract d)
  xs   [2048, 512] f16  = simp(b)
  wrep [128, 4, 128] f16: wrep[k,e,m] = w_o[e*128+k]  (stationary replicated
        along the PE output dim -> matmul broadcasts s_o to all partitions)
  wsbc [128, 512]  f16  = w_s replicated across partitions
  id128 [128, 128] f32  = identity (PE-transpose stationary)
  out  [2048, 2048] f16, host-upcast to f32.

The 2e-2 rel-err gate admits fp16 end to end (~1.5e-3 measured).
Per-core HBM traffic: 4.5 MiB in + 8.39 MiB out.

Engine schedule (evolved over ~10 profiled iterations):
  - PE: psum_so[p,i] = b + sum_d w_o[d]*orig[i,d] via 4 bias seeds + 16
    K=128/N=512 fp16 matmuls (w_rep stationary): the matmul does the
    reduction AND the 128-row broadcast, so s_o never materializes.
  - Row-blocks 0..11 (ScalarE): sigmoid ACTIVATE [128,2048] PSUM->SBUF
    f16, bias port adds s_s[t*128+p]; dense ~2us cadence.  Their simp rows
    load partition-inner so bias columns drop out of the DVE mul+reduce.
  - Row-blocks 12..15 (DVE): sigmoid(s) = 1/(1 + e^-(s_o+b)*e^-s_s).
    Their simp rows load partition-OUTER (4KB lines, first in queue); the
    [128,4] s_s matrix is interleaved with zero columns and PE-transposed
    (identity stationary) so after ScalarE's exp the zeros become the
    "ones" rows: each block's PSUM fill 1 + Es (x) Eo is then a single
    K=2 matmul per bank.  Two half-width PSUM tiles ping-pong so PE fill
    overlaps DVE's reciprocal_approx_fast; DVE casts f32->f16 and the
    stores ride the fast sync HWDGE queue (SWDGE cast-stores measured
    ~5us each and stalled the pipeline; a scatter DMA for es cost ~8us).
  - ACT table preloaded by a dep-free dummy sigmoid at t~0.
"""

import numpy as np

import concourse.mybir as mybir
from concourse import bacc, bass_utils
from concourse.tile import TileContext

P = 128          # partitions
D = 512          # feature dim
S = 2048         # sents
NT = S // P      # 16 row-tiles
NE = D // P      # 4 contraction chunks
NOFF = 4         # row-tiles 12..15 take the DVE/recip path
NSCA = NT - NOFF # row-tiles 0..11 take the ScalarE/sigmoid path
OGROUPS = [2, 4, 4, 1, 1]            # sync-store groups over tiles 0..11
NCORES = 8
F32 = mybir.dt.float32
F16 = mybir.dt.float16


def _kernel_body(tc, out, xot, xs, wrep, wsbc, id128, bvec):
    nc = tc.nc
    # simp rows for the ScalarE path, partition-inner: j = n*P + p
    xs_re = xs.rearrange("(n p) d -> p n d", p=P)
    # simp rows for the DVE path, partition-outer: j = NSCA*P + p*NOFF + c
    xso_re = xs[NSCA * P:S, :].rearrange("(p c) d -> p c d", c=NOFF)

    with (
        tc.tile_pool(name="consts", bufs=1) as cpool,
        tc.tile_pool(name="xin", bufs=1) as xpool,
        tc.tile_pool(name="scratch", bufs=3) as spool,
        tc.tile_pool(name="recb", bufs=3) as rpool,
        tc.tile_pool(name="outbuf", bufs=1) as opool,
        tc.tile_pool(name="psum", bufs=1, space="PSUM") as ppool,
    ):
        # preload the sigmoid ACT table while DMAs run: dummy activation
        # whose only deps are two DVE memsets, so it issues almost at t=0.
        dummy = cpool.tile([1, 1], F32, tag="dummy")
        dummy_b = cpool.tile([1, 1], F32, tag="dummyb")
        nc.vector.memset(dummy, 0.0)
        nc.vector.memset(dummy_b, 0.0)
        nc.scalar.activation(dummy, dummy,
                             mybir.ActivationFunctionType.Exp,
                             bias=dummy_b[:, 0:1])

        # tiny loads on the scalar HWDGE queue (empty early)
        wrep_sb = cpool.tile([P, NE, P], F16, tag="wrep")
        wsbc_sb = cpool.tile([P, D], F16, tag="wsbc")
        id_sb = cpool.tile([P, P], F32, tag="id128")
        b_sb = cpool.tile([1, 1], F32, tag="bsb")
        nc.scalar.dma_start(out=wrep_sb, in_=wrep)
        nc.scalar.dma_start(out=wsbc_sb, in_=wsbc)
        nc.scalar.dma_start(out=id_sb, in_=id128)
        nc.scalar.dma_start(out=b_sb, in_=bvec)

        ones_row = cpool.tile([1, P], F16, tag="ones")
        nc.vector.memset(ones_row, 1.0)
        b_row = cpool.tile([1, 512], F16, tag="brow")
        nc.vector.memset(b_row, 0.0)
        nc.vector.tensor_scalar_add(b_row, b_row, b_sb)
        # eo pair [e^-(s_o+b); 1] on adjacent partitions: one K=2 matmul
        # then computes 1 + Es*Eo straight into PSUM.  Row 1 memset now;
        # ScalarE's exp overwrites row 0 later.
        eo_ones = cpool.tile([2, S], F16, tag="eoones")
        nc.gpsimd.memset(eo_ones, 1.0)

        # --- input stream (sync queue, FIFO): xs_off -> xs[0:2] -> xot
        # e0..3 -> xs[2:4] -> xs[4:8] -> xs[8:12] ---
        xs_off = xpool.tile([P, NOFF, D], F16, tag="xsoff")
        nc.sync.dma_start(out=xs_off, in_=xso_re)
        xs_all = xpool.tile([P, NSCA, D], F16, tag="xs")
        nc.sync.dma_start(out=xs_all[:, 0:2, :], in_=xs_re[:, 0:2, :])
        xot_t = []
        for e in range(NE):
            xt = xpool.tile([P, S], F16, tag=f"xot{e}", name=f"xot{e}")
            nc.sync.dma_start(out=xt, in_=xot[e * P:(e + 1) * P, :])
            xot_t.append(xt)
        for lo, hi in ((2, 4), (4, 8), (8, 12)):
            nc.sync.dma_start(out=xs_all[:, lo:hi, :],
                              in_=xs_re[:, lo:hi, :])

        s_sb_mat = cpool.tile([P, NSCA], F32, tag="ssmat")  # s_s, col t
        s4_mat = cpool.tile([P, NOFF], F32, tag="s4mat")    # DVE-path s_s
        s4i = cpool.tile([P, 2 * NOFF], F32, tag="s4i")     # s_s cols | 0
        nc.vector.memset(s4i, 0.0)
        es2f = cpool.tile([2, NOFF * P], F32, tag="es2f")
        es2 = cpool.tile([2, NOFF * P], F16, tag="es2")     # [Es row; 1s]
        sob_psum = ppool.tile([P, S], F32, tag="sob")       # b + s_o
        # two half-width tiles ping-pong so PE fill overlaps DVE reciprocal
        pt_a = ppool.tile([P, S // 2], F32, tag="pta")
        pt_b = ppool.tile([P, S // 2], F32, tag="ptb")

        # --- PE: b seed, then accumulate w_o-weighted transposed orig ---
        for j in range(S // 512):
            nc.tensor.matmul(sob_psum[:, j * 512:(j + 1) * 512], ones_row,
                             b_row, start=True, stop=False)
        for e in range(NE):
            for j in range(S // 512):
                nc.tensor.matmul(sob_psum[:, j * 512:(j + 1) * 512],
                                 wrep_sb[:, e, :],
                                 xot_t[e][:, j * 512:(j + 1) * 512],
                                 start=False, stop=(e == NE - 1))

        # --- DVE: dots for the DVE-path rows first, interleaved with
        # zeros, then PE-transposed; ScalarE exps the result to f16 ---
        prodo = spool.tile([P, NOFF, D], F16, tag="prodo")
        for c in range(NOFF):
            nc.vector.tensor_mul(out=prodo[:, c, :], in0=xs_off[:, c, :],
                                 in1=wsbc_sb)
        nc.vector.tensor_reduce(
            s4_mat, prodo, axis=mybir.AxisListType.X, op=mybir.AluOpType.add)
        nc.vector.tensor_copy(
            out=s4i.rearrange("p (c two) -> p c two", two=2)[:, :, 0],
            in_=s4_mat)
        # four [128,2] -> [2,128] PE transposes (stationary base must be a
        # multiple of 32, so each pair lands at partitions 0-1 of its own
        # PSUM column range), then one DVE copy out
        for k in range(NOFF):
            nc.tensor.transpose(out=pt_a[0:2, k * P:(k + 1) * P],
                                in_=s4i[:, 2 * k:2 * k + 2],
                                identity=id_sb)
        nc.vector.tensor_copy(out=es2f, in_=pt_a[0:2, 0:NOFF * P])

        def simp_dots(g, lo, hi):
            prod = spool.tile([P, hi - lo, D], F16, tag="prod",
                              name=f"ps{g}")
            for blk in range(hi - lo):
                nc.vector.tensor_mul(out=prod[:, blk, :],
                                     in0=xs_all[:, lo + blk, :],
                                     in1=wsbc_sb)
            nc.vector.tensor_reduce(
                s_sb_mat[:, lo:hi], prod,
                axis=mybir.AxisListType.X, op=mybir.AluOpType.add)

        simp_dots(0, 0, 2)
        simp_dots(1, 2, 4)
        simp_dots(2, 4, 8)
        simp_dots(3, 8, 12)

        group_of_tile = []
        for gi, gsz in enumerate(OGROUPS):
            group_of_tile += [gi] * gsz
        group_start = np.cumsum([0] + OGROUPS).tolist()

        out_all = opool.tile([P, NT, S], F16, tag="oall")

        def sigmoid_tile(t):
            nc.scalar.activation(
                out_all[:, t, :], sob_psum,
                mybir.ActivationFunctionType.Sigmoid,
                bias=s_sb_mat[:, t:t + 1], scale=1.0)
            gi = group_of_tile[t]
            if t == group_start[gi] + OGROUPS[gi] - 1:
                t0_g = group_start[gi]
                gsz = OGROUPS[gi]
                r0 = t0_g * P
                if gsz == 1:
                    nc.sync.dma_start(out=out[r0:r0 + P, :],
                                      in_=out_all[:, t0_g, :])
                else:
                    dst = out[r0:r0 + gsz * P, :].rearrange(
                        "(q p) i -> p q i", p=P)
                    nc.sync.dma_start(out=dst,
                                      in_=out_all[:, t0_g:t0_g + gsz, :])

        # ScalarE: both exps back-to-back (the dummy preloaded the Exp
        # table; interleaving Exp and Sigmoid reloads the ACT table at
        # ~1.3us per switch), then a single switch into the sigmoid stream.
        nc.scalar.activation(es2, es2f,
                             mybir.ActivationFunctionType.Exp, scale=-1.0)
        nc.scalar.activation(eo_ones[0:1, :], sob_psum[0:1, :],
                             mybir.ActivationFunctionType.Exp, scale=-1.0)
        for t in range(0, 6):
            sigmoid_tile(t)

        # --- DVE path, tiles 12..15: K=2 matmul fills 1 + Es (x) Eo into
        # ping-pong half-PSUM tiles; DVE reciprocal + f16 cast; stores on
        # the sync queue, interleaved before the tail ScalarE groups ---
        H = S // 2
        for k in range(NOFF):
            t = NSCA + k
            srecip = rpool.tile([P, S], F32, tag="srecip", name=f"sr{k}")
            for h, pt in enumerate((pt_a, pt_b)):
                for j in range(H // 512):
                    c0 = h * H + j * 512
                    nc.tensor.matmul(pt[:, j * 512:(j + 1) * 512],
                                     es2[:, k * P:(k + 1) * P],
                                     eo_ones[:, c0:c0 + 512],
                                     start=True, stop=True)
                nc.vector.reciprocal_approx_fast(
                    out=srecip[:, h * H:(h + 1) * H], in_=pt)
            nc.vector.tensor_copy(out=out_all[:, t, :], in_=srecip)
            nc.sync.dma_start(out=out[t * P:(t + 1) * P, :],
                              in_=out_all[:, t, :])

        for t in range(6, NSCA):
            sigmoid_tile(t)


def build_program():
    nc = bacc.Bacc(
        "TRN2",
        debug=False,
        target_bir_lowering=False,
        num_devices=NCORES,
    )
    xot = nc.dram_tensor("xot", [D, S], F16, kind="ExternalInput").ap()
    xs = nc.dram_tensor("xs", [S, D], F16, kind="ExternalInput").ap()
    wrep = nc.dram_tensor("wrep", [P, NE, P], F16, kind="ExternalInput").ap()
    wsbc = nc.dram_tensor("wsbc", [P, D], F16, kind="ExternalInput").ap()
    id128 = nc.dram_tensor("id128", [P, P], F32, kind="ExternalInput").ap()
    bvec = nc.dram_tensor("bvec", [1, 1], F32, kind="ExternalInput").ap()
    out = nc.dram_tensor("out", [S, S], F16, kind="ExternalOutput").ap()
    with TileContext(nc) as tc:
        _kernel_body(tc, out, xot, xs, wrep, wsbc, id128, bvec)
    nc.compile()
    return nc


_PROGRAM = None


def _get_program():
    global _PROGRAM
    if _PROGRAM is None:
        _PROGRAM = build_program()
    return _PROGRAM


def make_in_maps(prop_state, W, b):
    prop = np.asarray(prop_state, dtype=np.float32).astype(np.float16)
    w = np.asarray(W, dtype=np.float32).reshape(2 * D).astype(np.float16)
    w_o, w_s = w[:D], w[D:]
    # wrep[k, e, m] = w_o[e*128 + k], replicated along m (PE output dim)
    wrep = np.ascontiguousarray(
        np.broadcast_to(w_o.reshape(NE, P).T[:, :, None], (P, NE, P)))
    wsbc = np.ascontiguousarray(np.broadcast_to(w_s[None, :], (P, D)))
    id128 = np.eye(P, dtype=np.float32)
    bv = np.ascontiguousarray(np.asarray(b, dtype=np.float32).reshape(1, 1))
    maps = []
    for i in range(NCORES):
        xot = np.ascontiguousarray(prop[i, :S].T)         # [512, 2048]
        xs = prop[i, S:].copy()                           # [2048, 512]
        # DVE-path rows: permute so the device's partition-outer load
        # (j = j0 + p*NOFF + c) followed by the PE transpose yields
        # s_s[j0 + c*128 + p] at es2[0, c*128+p]
        j0 = NSCA * P
        xs[j0:] = xs[j0:].reshape(NOFF, P, D).transpose(1, 0, 2).reshape(
            NOFF * P, D)
        maps.append({"xot": xot, "xs": np.ascontiguousarray(xs),
                     "wrep": wrep, "wsbc": wsbc, "id128": id128,
                     "bvec": bv})
    return maps


def kernel(A, prop_state, W, b, _trace=False):
    nc = _get_program()
    in_maps = make_in_maps(prop_state, W, b)
    res = bass_utils.run_bass_kernel_spmd(
        nc, in_maps, core_ids=list(range(NCORES)), trace=_trace)
    out = np.stack([res.results[i]["out"] for i in range(NCORES)], axis=0)
    if _trace:
        kernel.last_results = res
    return out.astype(np.float32)


# revision 33
# speedup vs baseline: 1.1905x; 1.0727x over previous
"""Trainium2 Bass kernel for nn_AlignModel.

Computes out[b, j, i] = sigmoid(simp[b,j]·w_s + orig[b,i]·w_o + bias) where
orig/simp are the two halves of prop_state[b] ([B, 2S, D] -> [B,S,D] each),
w_o = W[0,:D], w_s = W[0,D:].

Sharding: data-parallel over batch B=8 across the 8 NeuronCores.  Host-side
staging per core (layout only -- all compute is on device):
  xot  [512, 2048] f16  = orig(b).T          (d-major, so PE can contract d)
  xs   [2048, 512] f16  = simp(b)
  wrep [128, 4, 128] f16: wrep[k,e,m] = w_o[e*128+k]  (stationary replicated
        along the PE output dim -> matmul broadcasts s_o to all partitions)
  wsbc [128, 512]  f16  = w_s replicated across partitions
  out  [2048, 2048] f16, host-upcast to f32.

The 2e-2 rel-err gate admits fp16 end to end (sigmoid outputs in (0,1):
~5e-4 rel err; fp16-input dots with f32 accumulation: ~1e-4 score error).
Per-core HBM traffic: 4.45 MiB in + 8.39 MiB out.

Engine schedule (from trace iteration; engines run disjoint jobs):
  - PE: psum_so[p,i] = b + sum_d w_o[d]*orig[i,d] via 4 bias seeds + 16
    K=128/N=512 fp16 matmuls (w_rep stationary).  s_o never materializes;
    the matmul does the reduction AND the 128-row broadcast.
  - DVE: simp dots only (fp16 mul at 2x + batched 4-tile reduce at 1x)
    into s_sb_mat columns -> always ahead of ScalarE's 2us/tile cadence.
  - ScalarE: ONLY the 16 sigmoid ACTIVATEs, [128,2048] PSUM->SBUF f16,
    bias port adds s_s[t*128+p].  ACT table preloaded by a dep-free dummy.
  - Load order on the sync queue (FIFO): xs group 0 -> xot e=0..3 -> xs
    groups 1-3; stores follow.  All per-partition descriptor lines are
    >=4KB except xs (1KB, layout-forced); small chunks measured
    ~100-150 GB/s vs ~400 GB/s at 4KB.
"""

import numpy as np

import concourse.mybir as mybir
from concourse import bacc, bass_utils
from concourse.tile import TileContext

P = 128          # partitions
D = 512          # feature dim
S = 2048         # sents
NT = S // P      # 16 row-tiles
NE = D // P      # 4 contraction chunks
SCH = 4          # simp tiles per load group
NSC = NT // SCH
OGROUPS = [1, 1, 2, 4, 4, 2, 1, 1]   # output row-tiles per store
NCORES = 8
F32 = mybir.dt.float32
F16 = mybir.dt.float16


def _kernel_body(tc, out, xot, xs, wrep, wsbc, bvec):
    nc = tc.nc
    # simp half, partition-inner: j = n*P + p  (bias needs column layout)
    xs_re = xs.rearrange("(n p) d -> p n d", p=P)

    with (
        tc.tile_pool(name="consts", bufs=1) as cpool,
        tc.tile_pool(name="xin", bufs=1) as xpool,
        tc.tile_pool(name="scratch", bufs=3) as spool,
        tc.tile_pool(name="outbuf", bufs=1) as opool,
        tc.tile_pool(name="psum", bufs=1, space="PSUM") as ppool,
    ):
        # preload the sigmoid ACT table while DMAs run: dummy activation
        # whose only deps are two DVE memsets, so it issues almost at t=0.
        dummy = cpool.tile([1, 1], F32, tag="dummy")
        dummy_b = cpool.tile([1, 1], F32, tag="dummyb")
        nc.vector.memset(dummy, 0.0)
        nc.vector.memset(dummy_b, 0.0)
        nc.scalar.activation(dummy, dummy,
                             mybir.ActivationFunctionType.Sigmoid,
                             bias=dummy_b[:, 0:1])

        # tiny loads on the scalar HWDGE queue (empty early, so these land
        # well before the big sync-queue stream needs them)
        wrep_sb = cpool.tile([P, NE, P], F16, tag="wrep")
        wsbc_sb = cpool.tile([P, D], F16, tag="wsbc")
        b_sb = cpool.tile([1, 1], F32, tag="bsb")
        nc.scalar.dma_start(out=wrep_sb, in_=wrep)
        nc.scalar.dma_start(out=wsbc_sb, in_=wsbc)
        nc.scalar.dma_start(out=b_sb, in_=bvec)

        ones_row = cpool.tile([1, P], F16, tag="ones")
        nc.vector.memset(ones_row, 1.0)
        b_row = cpool.tile([1, 512], F16, tag="brow")
        nc.vector.memset(b_row, 0.0)
        nc.vector.tensor_scalar_add(b_row, b_row, b_sb)

        # --- input stream (sync queue, FIFO): xs g0, xot e0..3, xs g1..3 ---
        xs_all = xpool.tile([P, NT, D], F16, tag="xs")
        nc.sync.dma_start(out=xs_all[:, 0:SCH, :], in_=xs_re[:, 0:SCH, :])
        xot_t = []
        for e in range(NE):
            xt = xpool.tile([P, S], F16, tag=f"xot{e}", name=f"xot{e}")
            nc.sync.dma_start(out=xt, in_=xot[e * P:(e + 1) * P, :])
            xot_t.append(xt)
        for g in range(1, NSC):
            nc.sync.dma_start(out=xs_all[:, g * SCH:(g + 1) * SCH, :],
                              in_=xs_re[:, g * SCH:(g + 1) * SCH, :])

        s_sb_mat = cpool.tile([P, NT], F32, tag="ssmat")  # s_s, col t
        sob_psum = ppool.tile([P, S], F32, tag="sob")     # b + s_o, all rows

        # --- PE: b seed, then accumulate w_o-weighted transposed orig ---
        for j in range(S // 512):
            nc.tensor.matmul(sob_psum[:, j * 512:(j + 1) * 512], ones_row,
                             b_row, start=True, stop=False)
        for e in range(NE):
            for j in range(S // 512):
                nc.tensor.matmul(sob_psum[:, j * 512:(j + 1) * 512],
                                 wrep_sb[:, e, :],
                                 xot_t[e][:, j * 512:(j + 1) * 512],
                                 start=False, stop=(e == NE - 1))

        # --- simp dots (DVE) + sigmoid row-blocks (ScalarE) + stores ---
        group_of_tile = []
        for gi, gsz in enumerate(OGROUPS):
            group_of_tile += [gi] * gsz
        group_start = np.cumsum([0] + OGROUPS).tolist()

        out_all = opool.tile([P, NT, S], F16, tag="oall")
        for g in range(NSC):
            prod = spool.tile([P, SCH, D], F16, tag="prod", name=f"ps{g}")
            for blk in range(SCH):
                nc.vector.tensor_mul(out=prod[:, blk, :],
                                     in0=xs_all[:, g * SCH + blk, :],
                                     in1=wsbc_sb)
            nc.vector.tensor_reduce(
                s_sb_mat[:, g * SCH:(g + 1) * SCH], prod,
                axis=mybir.AxisListType.X, op=mybir.AluOpType.add)
            for blk in range(SCH):
                t = g * SCH + blk
                nc.scalar.activation(
                    out_all[:, t, :], sob_psum,
                    mybir.ActivationFunctionType.Sigmoid,
                    bias=s_sb_mat[:, t:t + 1],
                    scale=1.0,
                )
                gi = group_of_tile[t]
                if t == group_start[gi] + OGROUPS[gi] - 1:
                    t0_g = group_start[gi]
                    gsz = OGROUPS[gi]
                    r0 = t0_g * P
                    if gsz == 1:
                        nc.sync.dma_start(out=out[r0:r0 + P, :],
                                          in_=out_all[:, t0_g, :])
                    else:
                        dst = out[r0:r0 + gsz * P, :].rearrange(
                            "(q p) i -> p q i", p=P)
                        nc.sync.dma_start(out=dst,
                                          in_=out_all[:, t0_g:t0_g + gsz, :])


def build_program():
    nc = bacc.Bacc(
        "TRN2",
        debug=False,
        target_bir_lowering=False,
        num_devices=NCORES,
    )
    xot = nc.dram_tensor("xot", [D, S], F16, kind="ExternalInput").ap()
    xs = nc.dram_tensor("xs", [S, D], F16, kind="ExternalInput").ap()
    wrep = nc.dram_tensor("wrep", [P, NE, P], F16, kind="ExternalInput").ap()
    wsbc = nc.dram_tensor("wsbc", [P, D], F16, kind="ExternalInput").ap()
    bvec = nc.dram_tensor("bvec", [1, 1], F32, kind="ExternalInput").ap()
    out = nc.dram_tensor("out", [S, S], F16, kind="ExternalOutput").ap()
    with TileContext(nc) as tc:
        _kernel_body(tc, out, xot, xs, wrep, wsbc, bvec)
    nc.compile()
    return nc


_PROGRAM = None


def _get_program():
    global _PROGRAM
    if _PROGRAM is None:
        _PROGRAM = build_program()
    return _PROGRAM


def make_in_maps(prop_state, W, b):
    prop = np.asarray(prop_state, dtype=np.float32).astype(np.float16)
    w = np.asarray(W, dtype=np.float32).reshape(2 * D).astype(np.float16)
    w_o, w_s = w[:D], w[D:]
    # wrep[k, e, m] = w_o[e*128 + k], replicated along m (PE output dim)
    wrep = np.ascontiguousarray(
        np.broadcast_to(w_o.reshape(NE, P).T[:, :, None], (P, NE, P)))
    wsbc = np.ascontiguousarray(np.broadcast_to(w_s[None, :], (P, D)))
    bv = np.ascontiguousarray(np.asarray(b, dtype=np.float32).reshape(1, 1))
    maps = []
    for i in range(NCORES):
        xot = np.ascontiguousarray(prop[i, :S].T)         # [512, 2048]
        xs = np.ascontiguousarray(prop[i, S:])            # [2048, 512]
        maps.append({"xot": xot, "xs": xs, "wrep": wrep,
                     "wsbc": wsbc, "bvec": bv})
    return maps


def kernel(A, prop_state, W, b, _trace=False):
    nc = _get_program()
    in_maps = make_in_maps(prop_state, W, b)
    res = bass_utils.run_bass_kernel_spmd(
        nc, in_maps, core_ids=list(range(NCORES)), trace=_trace)
    out = np.stack([res.results[i]["out"] for i in range(NCORES)], axis=0)
    if _trace:
        kernel.last_results = res
    return out.astype(np.float32)
